# revision 39
# baseline (speedup 1.0000x reference)
"""Trainium2 Bass kernel for batched FK chain with tanh-MLP joint correction.

Math: per batch row,
    corr = tanh MLP_{7-15-15-7}(joints);  th = joints + off + corr
    M_j = DH(alpha_j, a_j, d_j, th_j);    out = (M_0 @ ... @ M_6)[:3, 3]
Factorization: M_j = A_j @ Rz(th_j) with A_j constant, and col 3 of M_6 is
constant, so the chain is 6 steps of (z-rotation + constant affine) on a
3-vector.

Distribution: pure data parallel, batch/8 = 32768 rows per NeuronCore.

Per-core pipeline (units of 128-col chunks, pipelined against each other):
  - host packs a feature-major image [128, 2048]: partition q = 64h+8k+g
    (16 batch groups x 7 features, 8 gap rows), free n = 128c+p,
    batch row b = 256p + 16c + 8h + g; fp16 copy for matmuls, fp32 copy
    (host range-reduced to [-pi, pi], offset folded in) for the angle path;
    MLP bias b1' = b1 - W1 @ off absorbs the offset for the MLP input.
    Two input DMAs total: front (weights+consts+x16 of unit0), rest.
  - 3 MLP layers as fp16 block-pattern matmuls on PE (tile_position
    quadrants), tanh on ACT with per-partition bias; hidden stays fp16
  - PE transpose-mode matmuls accumulate th = x.T + corr.T into fp32 PSUM.
    Each start/stop accumulation pair is kept ADJACENT: interleaving open
    accumulation groups drops the first write, and fp16 transposes do not
    accumulate at all (both verified on HW).
  - sincos: ST = Sin(th) directly (ACT Sin is ~exact on [-pi,pi], within
    ~1e-2 out to ~4.2, covering th in [-pi-1, pi+1]); CT = 1-2*Sin(th/2)^2
    (exact).  One table set (silu_and_others) serves Tanh+Sin via a
    doctored table map -> single ACT table load
  - chain of 6 (z-rot + const affine) steps as fp16 plane ops split
    DVE/GPSIMD (GPSIMD cannot run scalar_tensor_tensor -- walrus rejects
    it), fk-derived scalars baked as immediates (program recompiled if
    the non-joints inputs change; cached otherwise)
  - unit k's chain overlaps unit k+1's MLP; fp16 outputs DMA out per unit

Schedule tuning (this session, via TimelineSim engine-occupancy analysis;
engine busy: ACT ~16us > DVE ~13 > PE ~11 > Pool ~11 over a ~34us span):
  - tpc=4: transpose/Sin psum tiles of 4 chunks -> Sin pipelines against
    the L3-tanh/transpose ladder instead of waiting for all 8 chunks
  - stage_last: last unit's Sin/Square/CT emitted per plane-pair
    (high->low) so its chain starts ~1us earlier (shorter DVE tail)
  - front DMA split in halves at the L1 512-col slice boundary; PE
    warm-up tile memset moved to GPSIMD (clears the startup barrier
    ~700ns before DVE) and warm-up count tuned so the PE pstate ramp
    bridges exactly until the input DMA lands
  - chain output order xn,yn,zn (zn last) + chb=5 chain pool bufs
Rejected by measurement: merged h-half tanh (ACT throughput win but L2
start delay), 3-unit / asymmetric unit splits (9,7)...(12,4) (ladder +
op-size overheads beat tail savings), chain column-split ladders
(per-op overhead at [128,64] dominates), ACT-offloaded q1/q2 (adds a
cross-engine hop to the serial chain ladder).

Measured: rel err ~8.3e-4 vs fp32 reference; ~33.6us TimelineSim,
~39.8us/iter on HW via the on-device For_i loop (session baseline was
43.8us/iter, 35.2us TimelineSim).
"""

import os
import numpy as np

import concourse.bass as bass
import concourse.tile as tile
from concourse import bacc, mybir
from concourse import bass_utils

N_CORES = 8
B = 262144
BC = B // N_CORES            # 32768 rows per core
NCH = 16                     # total 128-col chunks per core

CFG = {
    "units": (8, 8),       # chunk counts per pipeline unit (sum = 16)
    "tpc": 4,              # chunks per transpose/Sin psum tile
    # engine assignment for chain ops (mid units / last unit)
    # ops: t1 t2 t3 t4 yr xn q1 q2 yn zn
    "pool_mid": ("t2", "t4", "q2"),
    "act_mid": (),
    "pool_last": ("t2", "t4", "q2"),
    "act_last": (),
    "sq_pool": False,      # combine SQ op on Pool
    "stage_last": True,    # plane-staged Sin/combine for the last unit
    "sq_act_last": True,   # last unit's SQ on ACT (Square) instead of DVE
    "chb": 5,              # chain pool bufs
    "add_eng": "dve",      # th=x+corr add: dve | pool | split | dvepool
    "split_out": False,    # last unit's out-DMA split across SP+ACT queues
    "l3w": 1024,           # L3 activation piece width (cols)
    "warmups": 5,          # PE warm-up matmul count (pstate ramp bridge)
    "front_split": False,  # x16(unit0) as its own DMA on the ACT HWDGE queue
    "front_halves": True,  # split front DMA at the L1 512-col slice boundary
                           # so L1 h0-slice0 starts ~1.2us earlier
    "chsplit": 1,          # chain column-split: independent ladders per unit,
                           # emitted step-interleaved to hide sem latency
    "merge_h": False,      # L1/L2: one 4-bank psum + one tanh across h-halves
}

F16 = mybir.dt.float16
F32 = mybir.dt.float32
AF = mybir.ActivationFunctionType
OP = mybir.AluOpType

# ---- constants blob column map -------------------------------------------
C_BIAS1, C_BIAS2, C_BIAS3 = 0, 1, 2
# chain per-joint constants for ACT-offloaded ops: dsa_j, cad_j
def _CJ(j, k):
    return 4 + 2 * j + k
NCONST = 16


def _build_host_data(inputs):
    joints = np.asarray(inputs["joints"], np.float32)
    fk = np.asarray(inputs["fk_params"], np.float32)
    W1 = np.asarray(inputs["W1"], np.float32)
    b1 = np.asarray(inputs["b1"], np.float32)
    W2 = np.asarray(inputs["W2"], np.float32)
    b2 = np.asarray(inputs["b2"], np.float32)
    W3 = np.asarray(inputs["W3"], np.float32)
    b3 = np.asarray(inputs["b3"], np.float32)

    off = fk[:, 3]
    b1p = b1 - W1 @ off
    x_off = joints + off[None, :]          # [B, 7] fp32
    # exact host range-reduction for the angle path: th = x_red + corr
    # stays within [-pi-1, pi+1], where ACT Sin is accurate enough
    x_red = (np.remainder(x_off + np.pi, 2 * np.pi) - np.pi).astype(np.float32)

    # --- per-core feature-major images ---
    # batch row b = 256*p + 16*c + 8*h + g; partition q = 64*h + 8*k + g
    # img[q, 128*c + p] = src[b, k]
    def mkimg(src, dtype):
        out = []
        for core in range(N_CORES):
            jc = src[core * BC:(core + 1) * BC]            # [32768, 7]
            arr = jc.reshape(128, 16, 2, 8, 7)             # [p, c, h, g, k]
            arr = arr.transpose(2, 4, 3, 1, 0)             # [h, k, g, c, p]
            img = np.zeros((2, 8, 8, 16, 128), np.float32)
            img[:, :7] = arr
            out.append(np.ascontiguousarray(img.reshape(128, 2048)).astype(dtype))
        return out
    imgs32 = mkimg(x_red, np.float32)
    imgs16 = mkimg(x_off, np.float16)

    # --- block-pattern weights (fp16), packed into one [128, 304] blob ---
    blob = np.zeros((128, 304), np.float16)
    # L1: lhsT1[64h+8k+g, 15g+j] = W1[j, k]  (cols 0:120)
    for h in (0, 1):
        for k in range(7):
            for g in range(8):
                blob[64 * h + 8 * k + g, 15 * g:15 * g + 15] = W1[:, k]
    # L2: lhsT2[15g+i, 15g+j] = W2[j, i]  (cols 120:240)
    for g in range(8):
        blob[15 * g:15 * g + 15, 120 + 15 * g:120 + 15 * g + 15] = W2.T
    # L3: lhsT3[15g+i, 8k+g] = W3[k, i]  (cols 240:304)
    for g in range(8):
        for k in range(7):
            blob[15 * g:15 * g + 15, 240 + 8 * k + g] = W3[k, :]

    # --- constants blob [128, NCONST] fp32 ---
    consts = np.zeros((128, NCONST), np.float32)
    for g in range(8):
        for j in range(15):
            consts[15 * g + j, C_BIAS1] = b1p[j]
            consts[15 * g + j, C_BIAS2] = b2[j]
    for h in (0, 1):
        for k in range(7):
            for g in range(8):
                consts[64 * h + 8 * k + g, C_BIAS3] = b3[k]
    alpha, a, d = fk[:, 0], fk[:, 1], fk[:, 2]
    ca, sa = np.cos(alpha), np.sin(alpha)
    for j in range(5):
        consts[:, _CJ(j, 0)] = d[j] * sa[j]
        consts[:, _CJ(j, 1)] = ca[j] * d[j]

    id32 = np.ascontiguousarray(np.eye(128, dtype=np.float32))
    cblob = np.concatenate([
        blob.view(np.uint8).reshape(128, 608),
        consts.view(np.uint8).reshape(128, NCONST * 4),
    ], axis=1)
    return imgs32, imgs16, np.ascontiguousarray(cblob), id32


def _emit_program(nc, sc, reps=1, loop_n=0):
    units = tuple(CFG["units"])
    assert sum(units) == NCH
    C1 = units[0]
    CR = NCH - C1
    CBYTES = 608 + NCONST * 4
    # front blob: weights+consts (and, when front_split is off, x16(unit0)).
    # With front_split, x16(unit0) rides its own DMA on the ACT HWDGE queue
    # so it transfers in parallel with the (tiny) weights DMA on SP.
    FSPLIT = CFG.get("front_split", False)
    # halves split requires the L1 512-col slice boundary to align
    FHALVES = CFG.get("front_halves", False) and not FSPLIT and C1 == 8
    XB = 256 * C1
    if FHALVES:
        FBYTES = CBYTES + XB // 2
    else:
        FBYTES = CBYTES + (0 if FSPLIT else XB)
    RBYTES = 256 * CR + 512 + 8192
    dfront = nc.dram_tensor("front", [128, FBYTES], mybir.dt.uint8,
                            kind="ExternalInput")
    if FSPLIT:
        dx0 = nc.dram_tensor("x0", [128, XB], mybir.dt.uint8,
                             kind="ExternalInput")
    if FHALVES:
        dfrontb = nc.dram_tensor("frontb", [128, XB // 2], mybir.dt.uint8,
                                 kind="ExternalInput")
    drest = nc.dram_tensor("rest", [128, RBYTES], mybir.dt.uint8,
                           kind="ExternalInput")
    dout = nc.dram_tensor("out", [128, 768], F16, kind="ExternalOutput")

    from contextlib import ExitStack, nullcontext
    with tile.TileContext(nc) as tc, ExitStack() as ctx:
        cp = ctx.enter_context(tc.tile_pool(name="persist", bufs=1))
        hp = ctx.enter_context(tc.tile_pool(name="halfp", bufs=2))
        merge_h = CFG.get("merge_h", False)
        if merge_h:
            # L1/L2 share one 4-bank psum tile per layer (both h-halves);
            # L3 gets its own 2-bank tile; tp shrinks to 1-bank tiles
            # (requires tpc<=4): 4 + 2 + 2x1 = 8 banks.
            assert CFG["tpc"] <= 4
            mlp_ps = ctx.enter_context(tc.tile_pool(name="mlpps", bufs=1, space="PSUM"))
            mlp3_ps = ctx.enter_context(tc.tile_pool(name="mlp3ps", bufs=1, space="PSUM"))
        else:
            # [128,1024] fp32 = 2 psum banks; bufs=2 -> 4 banks.  tp: 2x2 banks.
            mlp_ps = ctx.enter_context(tc.tile_pool(name="mlpps", bufs=2, space="PSUM"))
            mlp3_ps = mlp_ps
        tp_ps = ctx.enter_context(tc.tile_pool(name="tpps", bufs=2, space="PSUM"))
        chp = ctx.enter_context(tc.tile_pool(name="chain", bufs=CFG["chb"]))

        front = cp.tile([128, FBYTES], mybir.dt.uint8, tag="front")
        lhs = front[:, 0:608].bitcast(F16)
        consts = front[:, 608:608 + NCONST * 4].bitcast(F32)
        if FSPLIT:
            x0t = cp.tile([128, XB], mybir.dt.uint8, tag="x0t")
        if FHALVES:
            frontb = cp.tile([128, XB // 2], mybir.dt.uint8, tag="frontb")
        rest = cp.tile([128, RBYTES], mybir.dt.uint8, tag="rest")
        id32 = rest[:, 256 * CR:256 * CR + 512].bitcast(F32)
        ximg_all = rest[:, 256 * CR + 512:].bitcast(F32)

        # hoist the ACT table load under the input DMAs (memset on Pool --
        # it clears the startup barrier ~700ns before DVE does)
        warm = cp.tile([128, 1], F32, tag="warm")
        nc.gpsimd.memset(warm[:], 0.0)
        nc.scalar.activation(warm[:], warm[:], AF.Tanh, bias=0.0)

        def cv(col, parts=128):
            return consts[0:parts, col:col + 1]

        def mlp_half(xsl, hf, C):
            """3-layer tanh MLP for one chunk (C 128-col groups).
            ``xsl(h, so, sw)`` yields the L1 input slice.
            Returns corr [128, 128*C] fp32 (feature-major layout)."""
            nf = 128 * C
            h1 = hp.tile([128, 2 * nf], F16, tag=f"h1_{hf}", name="h1")
            h2 = hp.tile([128, 2 * nf], F16, tag=f"h2_{hf}", name="h2")
            corr = hp.tile([128, nf], F32, tag=f"corr_{hf}", name="corr")

            if merge_h:
                # L1: both h-halves into one 4-bank psum, single tanh
                ps = mlp_ps.tile([128, 2 * nf], F32, tag="l12ps", name="ps")
                for h in (0, 1):
                    for so in range(0, nf, 512):
                        sw = min(512, nf - so)
                        nc.tensor.matmul(
                            ps[0:120, nf * h + so:nf * h + so + sw],
                            lhs[64 * h:64 * h + 64, 0:120],
                            xsl(h, so, sw),
                            start=True, stop=True, tile_position=(64 * h, 0))
                nc.scalar.activation(
                    h1[0:120, 0:2 * nf],
                    ps[0:120, 0:2 * nf], AF.Tanh, bias=cv(C_BIAS1, 120))
                # L2
                ps = mlp_ps.tile([128, 2 * nf], F32, tag="l12ps", name="ps")
                for h in (0, 1):
                    col = nf * h
                    for so in range(0, nf, 512):
                        sw = min(512, nf - so)
                        nc.tensor.matmul(
                            ps[0:120, col + so:col + so + sw],
                            lhs[0:120, 120:240],
                            h1[0:120, col + so:col + so + sw],
                            start=True, stop=True)
                nc.scalar.activation(
                    h2[0:120, 0:2 * nf],
                    ps[0:120, 0:2 * nf], AF.Tanh, bias=cv(C_BIAS2, 120))
            else:
              # one psum tile + one activation per (layer, h-half, <=1024-col
              # block) -- blocks allow units wider than 8 chunks
              for h in (0, 1):
                for b0 in range(0, nf, 1024):
                  bw = min(1024, nf - b0)
                  ps = mlp_ps.tile([128, 1024], F32, tag="mlpps", name="ps")
                  for so in range(b0, b0 + bw, 512):
                    sw = min(512, b0 + bw - so)
                    nc.tensor.matmul(
                        ps[0:120, so - b0:so - b0 + sw],
                        lhs[64 * h:64 * h + 64, 0:120],
                        xsl(h, so, sw),
                        start=True, stop=True, tile_position=(64 * h, 0))
                  nc.scalar.activation(
                    h1[0:120, nf * h + b0:nf * h + b0 + bw],
                    ps[0:120, 0:bw], AF.Tanh, bias=cv(C_BIAS1, 120))
              # L2
              for h in (0, 1):
                col = nf * h
                for b0 in range(0, nf, 1024):
                  bw = min(1024, nf - b0)
                  ps = mlp_ps.tile([128, 1024], F32, tag="mlpps", name="ps")
                  for so in range(b0, b0 + bw, 512):
                    sw = min(512, b0 + bw - so)
                    nc.tensor.matmul(
                        ps[0:120, so - b0:so - b0 + sw],
                        lhs[0:120, 120:240],
                        h1[0:120, col + so:col + so + sw],
                        start=True, stop=True)
                  l2w = CFG.get("l2w", 1024)
                  for so in range(0, bw, l2w):
                    sw = min(l2w, bw - so)
                    nc.scalar.activation(
                      h2[0:120, col + b0 + so:col + b0 + so + sw],
                      ps[0:120, so:so + sw], AF.Tanh, bias=cv(C_BIAS2, 120))
            # L3: both h-halves stacked on psum partitions via col groups.
            # The activation is split at tp-tile boundaries so the corr
            # transposes (and Sins) can start before the whole half is done.
            for b0 in range(0, nf, 1024):
                bw = min(1024, nf - b0)
                ps = mlp3_ps.tile([128, 1024], F32,
                                  tag="mlp3ps" if merge_h else "mlpps", name="ps")
                for h in (0, 1):
                    col = nf * h
                    for so in range(b0, b0 + bw, 512):
                        sw = min(512, b0 + bw - so)
                        nc.tensor.matmul(
                            ps[64 * h:64 * h + 64, so - b0:so - b0 + sw],
                            lhs[0:120, 240:304],
                            h2[0:120, col + so:col + so + sw],
                            start=True, stop=True, tile_position=(0, 64 * h))
                for so in range(0, bw, CFG["l3w"]):
                    sw = min(CFG["l3w"], bw - so)
                    nc.scalar.activation(corr[:, b0 + so:b0 + so + sw],
                                         ps[:, so:so + sw], AF.Tanh,
                                         bias=cv(C_BIAS3))
            return corr

        def sincos_half(ximg, corr, hf, c0g, C, staged=False):
            """Transpose + accumulate th = x.T + corr.T, then
            ST = Sin(th), S2 = Sin(th/2), CT = 1 - 2*S2^2.
            Plane layout (k c m): plane j at cols [16*C*j : +16*C].
            staged=True: emit Sin/combine in plane-pair stages high->low so
            the chain (which consumes plane 5 first) starts sooner.
            Returns (CT, ST) [128, 16*C*6] fp16."""
            PL = 16 * C
            CT = hp.tile([128, 6 * PL], F16, tag=f"CT_{hf}", name="CT")
            ST = hp.tile([128, 6 * PL], F16, tag=f"ST_{hf}", name="ST")
            S2 = hp.tile([128, 6 * PL], F16, tag=f"S2_{hf}", name="S2")
            SQ = hp.tile([128, 6 * PL], F16, tag=f"SQ_{hf}", name="SQ")

            cbs = []
            o = 0
            while o < C:
                cw = min(CFG["tpc"], C - o)
                cbs.append((o, cw))
                o += cw
            stages = [(4, 6), (2, 4), (0, 2)] if staged else [(0, 6)]
            sq_act = staged_sq_act = CFG.get("sq_act_last") and hf == len(CFG["units"]) - 1
            sq_eng = nc.gpsimd if CFG["sq_pool"] else nc.vector
            # fp32 PSUM transposes accumulate th = x.T + corr.T (fp16
            # transposes bypass the accumulator -- verified on HW)
            pss = []
            for (o, cw) in cbs:
                ps2 = tp_ps.tile([128, 128 * cw], F32, tag="tpps", name="ps2")
                pss.append((ps2, o, cw))
                for cl in range(cw):
                    c = o + cl
                    # keep each accumulation group adjacent (start then stop)
                    # -- interleaving open groups drops the first write
                    nc.tensor.matmul(
                        ps2[:, 128 * cl:128 * cl + 128],
                        ximg[:, 128 * c:128 * c + 128], id32[:],
                        is_transpose=True, start=True, stop=False)
                    nc.tensor.matmul(
                        ps2[:, 128 * cl:128 * cl + 128],
                        corr[:, 128 * c:128 * c + 128], id32[:],
                        is_transpose=True, start=False, stop=True)
            for (k0, k1) in stages:
                for (ps2, o, cw) in pss:
                    # views: psum (c h k g) -> planes (k c m), m = 8h+g in 16
                    in_v = ps2[:, 0:128 * cw].rearrange(
                        "p (c h k g) -> p c h k g",
                        c=cw, h=2, k=8, g=8)[:, :, :, k0:k1, :]

                    def pv(t):
                        return t[:, :].rearrange(
                            "p (k c h g) -> p c h k g",
                            k=6, c=C, h=2, g=8)[:, o:o + cw, :, k0:k1, :]
                    nc.scalar.activation(pv(ST), in_v, AF.Sin, bias=0.0, scale=1.0)
                    nc.scalar.activation(pv(S2), in_v, AF.Sin, bias=0.0, scale=0.5)
                # CT = 1 - 2*S2^2 for this plane range
                lo, hi = PL * k0, PL * k1
                if sq_act:
                    nc.scalar.activation(SQ[:, lo:hi], S2[:, lo:hi],
                                         AF.Square, bias=0.0)
                else:
                    sq_eng.tensor_tensor(SQ[:, lo:hi], S2[:, lo:hi],
                                         S2[:, lo:hi], OP.mult)
                nc.vector.tensor_scalar(CT[:, lo:hi], SQ[:, lo:hi], -2.0, 1.0,
                                        OP.mult, OP.add)
            return CT, ST

        def chain_half(CT, ST, pack, uidx, PL, m0, is_last):
            pool_ops = set(CFG["pool_last" if is_last else "pool_mid"])
            act_ops = set(CFG["act_last" if is_last else "act_mid"])
            nsplit = max(1, int(CFG.get("chsplit", 1)))

            def eng(nm):
                if nm in pool_ops:
                    return nc.gpsimd
                return nc.vector

            # Column-split the unit's chain into `nsplit` independent
            # ladders, emitted step-interleaved (ladder A step j, ladder B
            # step j, A step j-1, ...) so each engine always has the other
            # ladder's ops queued behind a sem wait -- hides the per-step
            # cross-engine latency of the serial recurrence.
            w = PL // nsplit
            states = []
            for s in range(nsplit):
                lo = s * w

                def ctj(j, lo=lo):
                    return CT[:, PL * j + lo:PL * j + lo + w]

                def stj(j, lo=lo):
                    return ST[:, PL * j + lo:PL * j + lo + w]

                def ch(tag, s=s):
                    return chp.tile([128, w], F16,
                                    tag=f"{tag}{uidx}_{s}", name=tag)

                # step 5 init: x,y,z from ct5/st5
                u1 = ch("u1")
                nc.vector.tensor_scalar(u1, ctj(5), sc["s5u1m"], sc["s5u1a"], OP.mult, OP.add)
                x = ch("x")
                nc.vector.scalar_tensor_tensor(x, stj(5), sc["s5xm"], u1, OP.mult, OP.add)
                u3 = ch("u3")
                nc.vector.tensor_scalar(u3, stj(5), sc["s5u3m"], sc["s5u3a"], OP.mult, OP.add)
                y = ch("y")
                nc.vector.scalar_tensor_tensor(y, ctj(5), sc["s5ym"], u3, OP.mult, OP.add)
                u5 = ch("u5")
                nc.gpsimd.tensor_scalar(u5, stj(5), sc["s5u5m"], sc["s5u5a"], OP.mult, OP.add)
                z = ch("z")
                nc.vector.scalar_tensor_tensor(z, ctj(5), sc["s5zm"], u5, OP.mult, OP.add)
                states.append({"x": x, "y": y, "z": z, "lo": lo,
                               "ctj": ctj, "stj": stj, "ch": ch})

            for j in (4, 3, 2, 1, 0):
              for st_ in states:
                last = j == 0
                lo = st_["lo"]
                ctj, stj, ch = st_["ctj"], st_["stj"], st_["ch"]
                x, y, z = st_["x"], st_["y"], st_["z"]
                a_j, ca_j, sa_j = sc[f"a{j}"], sc[f"ca{j}"], sc[f"sa{j}"]
                dsa_j, cad_j = sc[f"dsa{j}"], sc[f"cad{j}"]
                t1 = ch("t1")
                eng("t1").tensor_tensor(t1, x, ctj(j), OP.mult)
                t2 = ch("t2")
                eng("t2").tensor_tensor(t2, y, stj(j), OP.mult)
                t3 = ch("t3")
                eng("t3").tensor_tensor(t3, x, stj(j), OP.mult)
                t4 = ch("t4")
                eng("t4").tensor_tensor(t4, y, ctj(j), OP.mult)
                # q1 = z*sa + d*sa ; q2 = z*ca + ca*d
                q1 = ch("q1")
                q2 = ch("q2")
                if "q1" in act_ops:
                    nc.scalar.activation(q1, z, AF.Identity,
                                         bias=cv(_CJ(j, 0)), scale=float(sa_j))
                else:
                    eng("q1").tensor_scalar(q1, z, sa_j, dsa_j, OP.mult, OP.add)
                if "q2" in act_ops:
                    nc.scalar.activation(q2, z, AF.Identity,
                                         bias=cv(_CJ(j, 1)), scale=float(ca_j))
                else:
                    eng("q2").tensor_scalar(q2, z, ca_j, cad_j, OP.mult, OP.add)
                yr = ch("yr")
                eng("yr").tensor_tensor(yr, t3, t4, OP.add)
                yn = pack[:, 256 + m0 + lo:256 + m0 + lo + w] if last else ch("y")
                zn = pack[:, 512 + m0 + lo:512 + m0 + lo + w] if last else ch("z")
                xn = pack[:, m0 + lo:m0 + lo + w] if last else ch("x")

                # Pool has no scalar_tensor_tensor -- use a 2-op TS+TT form
                def stt(nm, out, tin, s, op0, t2_, op1):
                    """out = (tin op0 s) op1 t2_"""
                    if nm in pool_ops:
                        tmp = ch(nm + "t")
                        nc.gpsimd.tensor_scalar(tmp, tin, s, None, op0)
                        nc.gpsimd.tensor_tensor(out, tmp, t2_, op1)
                    else:
                        nc.vector.scalar_tensor_tensor(out, tin, s, t2_,
                                                       op0, op1)
                stt("xn", xn, t1, a_j, OP.add, t2, OP.subtract)
                stt("yn", yn, yr, ca_j, OP.mult, q1, OP.subtract)
                stt("zn", zn, yr, sa_j, OP.mult, q2, OP.add)
                st_["x"], st_["y"], st_["z"] = xn, yn, zn

        # PE warm-up: dummy matmuls so the PE clock ramps during input DMAs.
        # The warm-up tile is memset on Pool (starts ~60ns; DVE memset sat
        # behind the startup barrier for ~800ns) and the count is sized to
        # bridge until the input DMAs land without delaying the first L1.
        wm16 = cp.tile([64, 512], F16, tag="wm16")
        nc.gpsimd.memset(wm16[:], 0.0)
        wmps = mlp3_ps.tile([128, 1024], F32,
                            tag="mlp3ps" if merge_h else "mlpps", name="wmps")
        for _w in range(CFG.get("warmups", 6)):
            nc.tensor.matmul(wmps[:, 0:512], wm16[0:64, 0:128],
                             wm16[0:64, :], start=True, stop=True)

        loop_ctx = tc.For_i(0, loop_n, 1) if loop_n else nullcontext()
        first = True
        with loop_ctx:
          for _rep in range(reps):
              if FSPLIT:
                  x16a = x0t[:, :].bitcast(F16)
              elif FHALVES:
                  x16a = front[:, CBYTES:FBYTES].bitcast(F16)  # cols 0:64*C1
                  x16b = frontb[:, :].bitcast(F16)             # cols 64*C1:
              else:
                  x16a = front[:, CBYTES:FBYTES].bitcast(F16)
              x16r = rest[:, 0:256 * CR].bitcast(F16)
              co = 0
              ximgs = []
              for k, C in enumerate(units):
                  ximgs.append(ximg_all[:, 128 * co:128 * (co + C)])
                  co += C
              pack = cp.tile([128, 768], F16, tag="pack", name="pack")
              # two input DMAs: front (weights+consts+x16 unit0) gates the
              # MLP; rest (x16 rest ++ id32 ++ fp16 angle image) follows
              if first:
                  nc.sync.dma_start(front[:], dfront.ap())
                  if FHALVES:
                      nc.sync.dma_start(frontb[:], dfrontb.ap())
                  first = False
              if FSPLIT:
                  # x16(unit0) on the ACT HWDGE queue: transfers in parallel
                  # with the weights DMA, so L1 can start ~0.5us earlier
                  nc.scalar.dma_start(x0t[:], dx0.ap())
              nc.sync.dma_start(rest[:], drest.ap())
              pv = pack[:, :].rearrange("p (c h) -> p c h", c=3, h=256)
              dv = dout.ap().rearrange("p (c h) -> p c h", c=3, h=256)
              # emission interleave: chain(k-1) lands between mlp(k) and
              # sincos(k) so the DVE queue order is combine0, chain0,
              # combine1, chain1, ...
              co = 0
              pend = None
              for k, C in enumerate(units):
                  if k == 0:
                      if FHALVES:
                          hc = 64 * C1  # fp16 cols in the front half

                          def xsl(h, so, sw, hc=hc):
                              t, o = (x16a, so) if so < hc else (x16b, so - hc)
                              assert o + sw <= hc
                              return t[64 * h:64 * h + 64, o:o + sw]
                      else:
                          def xsl(h, so, sw, t=x16a):
                              return t[64 * h:64 * h + 64, so:so + sw]
                  else:
                      x16u = x16r[:, 128 * (co - C1):128 * (co - C1 + C)]

                      def xsl(h, so, sw, t=x16u):
                          return t[64 * h:64 * h + 64, so:so + sw]
                  corr = mlp_half(xsl, k, C)
                  if pend is not None:
                      chain_half(*pend)
                      m0p, PLp = pend[5], pend[4]
                      nc.sync.dma_start(dv[:, :, m0p:m0p + PLp],
                                        pv[:, :, m0p:m0p + PLp])
                  last = k == len(units) - 1
                  staged = (last and CFG["stage_last"]) or (
                      not last and CFG.get("stage_mid", False))
                  CT, ST = sincos_half(ximgs[k], corr, k, 0, C,
                                       staged=staged)
                  pend = (CT, ST, pack, k, 16 * C, 16 * co, last)
                  co += C
              chain_half(*pend)
              m0p, PLp = pend[5], pend[4]
              if CFG["split_out"]:
                  # last unit: split by component across the two HWDGE
                  # queues -- x/y (written before z) go out via the idle
                  # ACT queue while z's descriptor generates on SP
                  nc.scalar.dma_start(dv[:, 0:2, m0p:m0p + PLp],
                                      pv[:, 0:2, m0p:m0p + PLp])
                  nc.sync.dma_start(dv[:, 2:3, m0p:m0p + PLp],
                                    pv[:, 2:3, m0p:m0p + PLp])
              else:
                  nc.sync.dma_start(dv[:, :, m0p:m0p + PLp],
                                    pv[:, :, m0p:m0p + PLp])


_PROG_CACHE = {}


def _baked_scalars(inputs):
    fk = np.asarray(inputs["fk_params"], np.float32)
    alpha, a, d = fk[:, 0], fk[:, 1], fk[:, 2]
    ca, sa = np.cos(alpha), np.sin(alpha)
    t6 = np.array([a[6], -d[6] * sa[6], ca[6] * d[6]], np.float32)
    sc = {
        "s5u1m": a[6], "s5u1a": a[5], "s5xm": -t6[1],
        "s5u3m": ca[5] * a[6], "s5u3a": -sa[5] * t6[2] - d[5] * sa[5],
        "s5ym": ca[5] * t6[1],
        "s5u5m": sa[5] * a[6], "s5u5a": ca[5] * t6[2] + ca[5] * d[5],
        "s5zm": sa[5] * t6[1],
    }
    for j in range(5):
        sc[f"a{j}"] = a[j]
        sc[f"ca{j}"] = ca[j]
        sc[f"sa{j}"] = sa[j]
        sc[f"dsa{j}"] = d[j] * sa[j]
        sc[f"cad{j}"] = ca[j] * d[j]
    return {k: float(np.float32(v)) for k, v in sc.items()}


def _get_program(inputs, reps=1, loop_n=0):
    sc = _baked_scalars(inputs)
    key = (tuple(sorted(sc.items())), reps, loop_n, tuple(sorted(CFG.items())))
    if key in _PROG_CACHE:
        return _PROG_CACHE[key]
    nc = bacc.Bacc("TRN2", target_bir_lowering=False, debug=False,
                   enable_asserts=False)
    _emit_program(nc, sc, reps=reps, loop_n=loop_n)

    # Force Tanh and Sin to resolve to the one table set containing both
    # (silu_and_others), so the kernel pays a single ACT table load.
    import concourse.bacc as bacc_mod
    from concourse.hw_specs import get_activation_tables
    orig_fn = bacc_mod.get_activation_tables
    tabs = get_activation_tables(nc.m.arch)
    trig = {AF.Tanh, AF.Sin}
    doctored = {
        name: (set(funcs) if name == "silu_and_others" else set(funcs) - trig)
        for name, funcs in tabs.items()
    }
    bacc_mod.get_activation_tables = lambda arch: doctored
    try:
        nc.compile()
    finally:
        bacc_mod.get_activation_tables = orig_fn

    _PROG_CACHE[key] = nc
    return nc


LAST_RESULTS = None  # BassKernelResults of the most recent run (for test.py)


def _host_in_maps(inputs):
    imgs32, imgs16, cblob, id32 = _build_host_data(inputs)
    C1 = CFG["units"][0]
    fsplit = CFG.get("front_split", False)
    fhalves = (CFG.get("front_halves", False) and not fsplit
               and CFG["units"][0] == 8)
    in_maps = []
    for core in range(N_CORES):
        x16 = imgs16[core]
        x0 = np.ascontiguousarray(x16[:, 0:128 * C1]).view(np.uint8)
        if fsplit:
            front = cblob
        elif fhalves:
            front = np.concatenate([cblob, x0[:, :128 * C1]], axis=1)
        else:
            front = np.concatenate([cblob, x0], axis=1)
        rest = np.concatenate(
            [np.ascontiguousarray(x16[:, 128 * C1:]).view(np.uint8),
             id32.view(np.uint8).reshape(128, 512),
             imgs32[core].view(np.uint8)], axis=1)
        m = {
            "front": np.ascontiguousarray(front),
            "rest": np.ascontiguousarray(rest),
        }
        if fsplit:
            m["x0"] = np.ascontiguousarray(x0)
        if fhalves:
            m["frontb"] = np.ascontiguousarray(x0[:, 128 * C1:])
        in_maps.append(m)
    return in_maps


def _jit_runner(nc):
    import jax
    from jax.sharding import Mesh, PartitionSpec, NamedSharding
    from jax.experimental.shard_map import shard_map
    from concourse import bass2jax
    bass2jax.install_neuronx_cc_hook()

    partition_name = nc.partition_id_tensor.name if nc.partition_id_tensor else None
    in_names, out_names, out_avals = [], [], []
    for alloc in nc.m.functions[0].allocations:
        if not isinstance(alloc, mybir.MemoryLocationSet):
            continue
        name = alloc.memorylocations[0].name
        if alloc.kind == "ExternalInput":
            if name != partition_name:
                in_names.append(name)
        elif alloc.kind == "ExternalOutput":
            out_names.append(name)
            out_avals.append(jax.core.ShapedArray(
                tuple(alloc.tensor_shape), mybir.dt.np(alloc.dtype)))
    all_in = in_names + out_names + ([partition_name] if partition_name else [])
    devices = jax.devices()[:N_CORES]
    mesh = Mesh(np.asarray(devices), ("core",))
    sh = NamedSharding(mesh, PartitionSpec("core"))

    def _body(*args):
        ops = list(args)
        if partition_name:
            ops.append(bass2jax.partition_id_tensor())
        outs = bass2jax._bass_exec_p.bind(
            *ops, out_avals=tuple(out_avals), in_names=tuple(all_in),
            out_names=tuple(out_names), lowering_input_output_aliases=(),
            sim_require_finite=True, sim_require_nnan=True, nc=nc)
        return tuple(outs)

    specs = (PartitionSpec("core"),) * (len(in_names) + len(out_names))
    ospec = (PartitionSpec("core"),) * len(out_names)
    f = jax.jit(shard_map(_body, mesh=mesh, in_specs=specs, out_specs=ospec,
                          check_rep=False))
    return f, in_names, out_avals, sh


def time_on_hw(inputs, n_lo=16, n_hi=256, iters=10):
    """Per-kernel HW time via an on-device For_i loop: slope of min wall
    between trip counts (includes ~2us loop back-edge per iteration)."""
    import time as _time
    import jax
    in_maps = _host_in_maps(inputs)
    mins = {}
    for loop_n in (n_lo, n_hi):
        nc = _get_program(inputs, loop_n=loop_n)
        f, in_names, out_avals, sh = _jit_runner(nc)
        cat = lambda n: np.concatenate(
            [np.asarray(in_maps[c][n]) for c in range(N_CORES)], axis=0)
        ci = [jax.device_put(cat(n), sh) for n in in_names]
        cz = [jax.device_put(
            np.zeros((N_CORES * a.shape[0], *a.shape[1:]), a.dtype), sh)
            for a in out_avals]
        jax.block_until_ready(f(*ci, *cz))
        best = float("inf")
        for _ in range(iters):
            t0 = _time.perf_counter()
            jax.block_until_ready(f(*ci, *cz))
            best = min(best, _time.perf_counter() - t0)
        mins[loop_n] = best
        print(f"[hw timing] loop_n={loop_n}: min wall {best*1e3:.2f} ms")
    slope_ns = (mins[n_hi] - mins[n_lo]) / (n_hi - n_lo) * 1e9
    print(f"[hw timing] -> {slope_ns:.0f} ns/kernel (incl ~2us loop overhead)")
    return slope_ns


def kernel(**inputs):
    global LAST_RESULTS
    j = np.asarray(inputs["joints"])
    assert j.shape == (B, 7), f"kernel hardcodes joints shape {(B, 7)}, got {j.shape}"
    nc = _get_program(inputs)
    in_maps = _host_in_maps(inputs)
    res = bass_utils.run_bass_kernel_spmd(nc, in_maps, core_ids=list(range(N_CORES)))
    LAST_RESULTS = res

    out = np.empty((B, 3), np.float32)
    for core in range(N_CORES):
        p = res.results[core]["out"].astype(np.float32)   # [128, 768] fp16
        # pack cols: [px(256) | py(256) | pz(256)], b_local = 256*p + m
        oc = p.reshape(128, 3, 256).transpose(0, 2, 1).reshape(BC, 3)
        out[core * BC:(core + 1) * BC] = oc
    return out

